# revision 8
# baseline (speedup 1.0000x reference)
"""Trainium2 Bass kernel v2 for nn_BertSelfAttention_43267500540531.

BertSelfAttention with relative-position key bias and relative-position value
aggregation (half-width 64), B=1, N=2048, HID=1024, 16 heads of d_head=64.

Sharding: 16 heads over 8 cores (2 heads/core). Each core gets full
hidden_states (bf16) and its 128-column slices of Wq/Wk/Wv, writes its 128
output columns; host concatenates.

v2 design (all on-chip, no DRAM skew bounce):
  - xT via x-bar DMA transposes; qT/kT projections per 512-quarter
  - scores computed transposed per j-block: sT[j, i] in a 4-bank psum tile
    [128, 2048]; one Exp activation per block reads the whole span
  - rel-k bias: a_k[i, w] (PE) -> gpsimd local_scatter skews each i-chunk
    into B[i', jj] = bias[i, j] -> three stationary-B matmuls against
    identity transpose-add the [j, i] windows into the score psum
  - softmax denominator L via ones-column appended to v
  - ctx reoriented: out[i-block, 65] accumulates 16 stationary-et matmuls
    (moving operand v_aug [128, 65]); normalization by 1/L on DVE writes
    the final [i, d] layout directly (no output transpose)
  - rel-v: band scores recomputed directly in [i, j] orientation (one small
    matmul per i-chunk + bias add via identity matmul), Exp, local_scatter
    to U[i', r], PE-transpose to U^T, two matmuls against W_rel_v chunks
    accumulate into the ctx psum group

The attention_mask is all-ones and the q/k/v biases are zero in this
problem's setup_inputs; both are validated at entry.
"""

import sys
from contextlib import ExitStack

for _p in ("/opt/trn_rl_repo", "/root/.axon_site/_ro/trn_rl_repo"):
    if _p not in sys.path:
        sys.path.append(_p)

import ml_dtypes
import numpy as np

import concourse.bacc as bacc
import concourse.mybir as mybir
import concourse.tile as tile
from concourse import bass_utils, library_config
from concourse.masks import make_identity

F32 = mybir.dt.float32
BF16 = mybir.dt.bfloat16
I16 = mybir.dt.int16
AF = mybir.ActivationFunctionType
BF = ml_dtypes.bfloat16

N = 2048
HID = 1024
DH = 64
HPC = 2
DPC = HPC * DH
NB = N // 128
NC8 = HID // 128
NCORES = 8
WBAND = 129
WPAD = 132
SCALE = 0.125
BW = 260  # bias scatter dst width (i'+w <= 127+131=258)


def build_kernel(nc, tc, ctx: ExitStack):
    xb = nc.dram_tensor("xb", [N, HID], BF16, kind="ExternalInput").ap()
    wqp = nc.dram_tensor("wqp", [128, HID], BF16, kind="ExternalInput").ap()
    wkp = nc.dram_tensor("wkp", [128, HID], BF16, kind="ExternalInput").ap()
    wvp = nc.dram_tensor("wvp", [128, HID], BF16, kind="ExternalInput").ap()
    wrkp = nc.dram_tensor("wrkp", [128, WPAD], BF16, kind="ExternalInput").ap()
    wrvp = nc.dram_tensor("wrvp", [128, 65], BF16, kind="ExternalInput").ap()
    idxbias = nc.dram_tensor("idxbias", [128, WPAD], I16, kind="ExternalInput").ap()
    idxb_m = nc.dram_tensor("idxb_m", [128, 512], I16, kind="ExternalInput").ap()
    idxb_0 = nc.dram_tensor("idxb_0", [128, 512], I16, kind="ExternalInput").ap()
    idxb_f = nc.dram_tensor("idxb_f", [128, 512], I16, kind="ExternalInput").ap()
    out = nc.dram_tensor("out", [N, DPC], F32, kind="ExternalOutput").ap()

    const_pool = ctx.enter_context(tc.tile_pool(name="const", bufs=1))
    qkT_pool = ctx.enter_context(tc.tile_pool(name="qkT", bufs=4))
    v_pool = ctx.enter_context(tc.tile_pool(name="vsb", bufs=NB))
    et_pool = ctx.enter_context(tc.tile_pool(name="expT", bufs=20))
    ctxp_pool = ctx.enter_context(tc.tile_pool(name="ctxp", bufs=32))
    ak_pool = ctx.enter_context(tc.tile_pool(name="ak", bufs=4))
    b_pool = ctx.enter_context(tc.tile_pool(name="bias", bufs=32))
    eb_pool = ctx.enter_context(tc.tile_pool(name="eb", bufs=3))
    u_pool = ctx.enter_context(tc.tile_pool(name="u", bufs=3))
    ut_pool = ctx.enter_context(tc.tile_pool(name="ut", bufs=34))
    out_pool = ctx.enter_context(tc.tile_pool(name="outsb", bufs=4))
    small_pool = ctx.enter_context(tc.tile_pool(name="small", bufs=4))

    ps_s_pool = ctx.enter_context(tc.tile_pool(name="ps_s", bufs=2, space="PSUM"))
    med_pool = ctx.enter_context(tc.tile_pool(name="ps_m", bufs=2, space="PSUM"))
    ctx_pool = ctx.enter_context(tc.tile_pool(name="ps_c", bufs=2, space="PSUM"))

    nc.gpsimd.load_library(library_config.local_scatter)

    identity = const_pool.tile([128, 128], F32, tag="ident")
    make_identity(nc, identity[:, :])
    identity_bf = const_pool.tile([128, 128], BF16, tag="identb")
    nc.vector.tensor_copy(identity_bf[:, :], identity[:, :])
    zero64 = const_pool.tile([128, 128], BF16, tag="z64")
    nc.vector.memset(zero64[:, :], 0.0)
    warm = const_pool.tile([1, 4], F32, tag="warm")
    nc.scalar.activation(warm[:, :], identity[0:1, 0:4], AF.Exp)

    # q/k weights first (needed by the first projection chunks)
    w_bf = {}
    for wname, wsrc in (("q", wqp), ("k", wkp)):
        wb = const_pool.tile([128, HID], BF16, tag=f"w{wname}")
        nc.sync.dma_start(wb[:, :], wsrc[:, :])
        w_bf[wname] = wb

    # xT via x-bar DMA transposes. First j-quarter split per chunk so the
    # first qT pass can start early; remaining 3 quarters in one DMA each.
    xT_ctx = ExitStack()
    xT_pool = xT_ctx.enter_context(tc.tile_pool(name="xT", bufs=NC8))
    xT_t = [[xT_pool.tile([128, 512], BF16, tag=f"xTq{q}", name=f"xT{q}_{i}")
             for i in range(NC8)] for q in range(2)]
    xT_cd = [xT_pool.tile([128, 1024], BF16, tag="xTcd", name=f"xTcd{i}")
             for i in range(NC8)]
    wrk_bf = const_pool.tile([128, WPAD], BF16, tag="wrk")
    nc.sync.dma_start(wrk_bf[:, :], wrkp[:, :])
    ixb = const_pool.tile([128, WPAD], I16, tag="ixb")
    nc.sync.dma_start(ixb[:, :], idxbias[:, :])
    for q in range(2):
        for chunk in range(NC8):
            cs = slice(chunk * 128, (chunk + 1) * 128)
            nc.sync.dma_start_transpose(
                xT_t[q][chunk][:, :], xb[q * 512 : (q + 1) * 512, cs]
            )
    for chunk in range(NC8):
        cs = slice(chunk * 128, (chunk + 1) * 128)
        nc.sync.dma_start_transpose(xT_cd[chunk][:, :], xb[1024:N, cs])

    def xT_mov(chunk, c0, c1):
        """moving operand slice of x^T chunk covering token cols [c0, c1)"""
        if c1 <= 1024:
            q = c0 // 512
            assert c1 <= (q + 1) * 512
            return xT_t[q][chunk][:, c0 - q * 512 : c1 - q * 512]
        assert c0 >= 1024
        return xT_cd[chunk][:, c0 - 1024 : c1 - 1024]

    # remaining constant loads (issued behind the xT transposes)
    wb = const_pool.tile([128, HID], BF16, tag="wv")
    nc.sync.dma_start(wb[:, :], wvp[:, :])
    w_bf["v"] = wb
    wrv_sb = const_pool.tile([128, 65], BF16, tag="wrv")
    nc.sync.dma_start(wrv_sb[:, :], wrvp[:, :])
    ixm = const_pool.tile([128, 512], I16, tag="ixm")
    nc.sync.dma_start(ixm[:, :], idxb_m[:, :])
    ix0 = const_pool.tile([128, 512], I16, tag="ix0")
    nc.sync.dma_start(ix0[:, :], idxb_0[:, :])
    ixf = const_pool.tile([128, 512], I16, tag="ixf")
    nc.sync.dma_start(ixf[:, :], idxb_f[:, :])

    # qT / kT quarters
    qTq = [qkT_pool.tile([128, 512], BF16, tag="qT", name=f"qT{q}")
           for q in range(4)]
    kTq = [qkT_pool.tile([128, 512], BF16, tag="kT", name=f"kT{q}")
           for q in range(4)]

    def emit_proj(ps, off, dst, wname, q):
        pslice = ps[:, off : off + 512]
        for chunk in range(NC8):
            nc.tensor.matmul(
                pslice,
                w_bf[wname][:, chunk * 128 : (chunk + 1) * 128],
                xT_mov(chunk, q * 512, (q + 1) * 512),
                start=(chunk == 0),
                stop=(chunk == NC8 - 1),
                skip_group_check=True,
            )
        nc.vector.tensor_copy(dst[:, :], pslice)

    ak_sb = {}
    B = {}

    def emit_ak(h, ib):
        hs = h * DH
        pa = ctx_pool.tile([128, WPAD], F32, tag="ctx", name=f"pa{h}_{ib}")
        nc.tensor.matmul(
            pa[:, :],
            qTq[ib // 4][hs : hs + DH, (ib % 4) * 128 : (ib % 4 + 1) * 128],
            wrk_bf[hs : hs + DH, :],
            start=True,
            stop=True,
        )
        ak = ak_pool.tile([128, WPAD], BF16, tag="ak", name=f"ak{h}_{ib}")
        nc.vector.tensor_copy(ak[:, :], pa[:, :])
        ak_sb[(h, ib)] = ak
        Bt = b_pool.tile([128, BW], BF16, tag="bias", name=f"B{h}_{ib}")
        nc.gpsimd.local_scatter(
            Bt[:, :], ak[:, :], ixb[:, :],
            channels=128, num_elems=BW, num_idxs=WPAD,
        )
        B[(h, ib)] = Bt

    v_sb = []

    def emit_v(jb):
        pv = ctx_pool.tile([128, DPC], F32, tag="ctx", name=f"pv{jb}")
        for chunk in range(NC8):
            nc.tensor.matmul(
                pv[:, :],
                xT_mov(chunk, jb * 128, (jb + 1) * 128),
                w_bf["v"][:, chunk * 128 : (chunk + 1) * 128],
                start=(chunk == 0),
                stop=(chunk == NC8 - 1),
            )
        vt = v_pool.tile([128, 130], BF16, tag="vsb", name=f"vsb{jb}")
        nc.vector.tensor_copy(vt[:, 0:64], pv[:, 0:64])
        nc.vector.tensor_copy(vt[:, 65:129], pv[:, 64:128])
        nc.vector.memset(vt[:, 64:65], 1.0)
        nc.vector.memset(vt[:, 129:130], 1.0)
        v_sb.append(vt)

    def kT_mov(hs, j0, j1):
        """pieces (col, width, ap) of kT rows [hs:hs+64] over cols [j0, j1)"""
        pieces = []
        j = j0
        while j < j1:
            q = j // 512
            je = min(j1, (q + 1) * 512)
            pieces.append(
                (j, je - j, kTq[q][hs : hs + DH, j - q * 512 : je - q * 512])
            )
            j = je
        return pieces

    def emit_band_pair(h, icp):
        """two band windows (ic0, ic0+1) -> one exp -> one scatter U2"""
        hs = h * DH
        ic0 = 2 * icp
        ps_b = med_pool.tile([128, 512], F32, tag="med", name=f"psb{h}_{icp}")
        for sub in range(2):
            ic = ic0 + sub
            base = sub * 256
            w0 = ic * 128 - 64
            edge = ic == 0 or ic == NB - 1
            if edge:
                nc.tensor.matmul(
                    ps_b[:, base : base + 256], zero64[hs : hs + DH, :],
                    kTq[0][hs : hs + DH, 0:256],
                    start=True, stop=False, skip_group_check=True,
                )
            first = not edge
            for j, width, mov in kT_mov(hs, max(0, w0), min(N, w0 + 256)):
                nc.tensor.matmul(
                    ps_b[:, base + j - w0 : base + j - w0 + width],
                    qTq[ic // 4][hs : hs + DH,
                                 (ic % 4) * 128 : (ic % 4 + 1) * 128],
                    mov,
                    start=first,
                    stop=False,
                    skip_group_check=True,
                )
                first = False
            nc.tensor.matmul(
                ps_b[:, base : base + 256], identity_bf[:, :],
                B[(h, ic)][:, 0:256],
                start=False, stop=(sub == 1), skip_group_check=True,
            )
        eb = eb_pool.tile([128, 512], BF16, tag="eb", name=f"eb{h}_{icp}")
        nc.scalar.activation(eb[:, :], ps_b[:, :], AF.Exp, scale=SCALE)
        U = u_pool.tile([128, 512], BF16, tag="u", name=f"u{h}_{icp}")
        ix = ix0 if icp == 0 else (ixf if icp == NB // 2 - 1 else ixm)
        nc.gpsimd.local_scatter(
            U[:, :], eb[:, :], ix[:, :],
            channels=128, num_elems=512, num_idxs=512,
        )
        return U

    UT = {}

    def emit_ut_pair(h, icp, U):
        pt = med_pool.tile([128, 256], BF16, tag="med", name=f"pt{h}_{icp}")
        nc.tensor.matmul(pt[:, 0:128], U[:, 0:128], identity_bf[:, :],
                         is_transpose=True)
        nc.tensor.matmul(pt[:, 128:256], U[:, 256:384], identity_bf[:, :],
                         is_transpose=True)
        ut = ut_pool.tile([128, 256], BF16, tag="ut", name=f"ut{h}_{icp}")
        nc.vector.tensor_copy(ut[:, :], pt[:, :])
        UT[(h, 2 * icp)] = ut[:, 0:128]
        UT[(h, 2 * icp + 1)] = ut[:, 128:256]

    UT = {}

    def emit_ut(h, ic, U):
        pt = med_pool.tile([128, 128], BF16, tag="med", name=f"pt{h}_{ic}")
        nc.tensor.matmul(pt[:, 0:128], U[:, 0:128], identity_bf[:, :],
                         is_transpose=True)
        ut = ut_pool.tile([128, 128], BF16, tag="ut", name=f"ut{h}_{ic}")
        nc.vector.tensor_copy(ut[:, :], pt[:, 0:128])
        UT[(h, ic)] = ut

    def _score_pieces(h, jc, i0):
        """bias pieces for jc whose target col lies in [i0, i0+1024)"""
        specs = [((slice(0, 64), (jc - 1) * 128 + 64), jc - 1,
                  (slice(64, 128), slice(192, 256)),
                  identity_bf[64:128, 64:128]),
                 ((slice(64, 128), (jc + 1) * 128), jc + 1,
                  (slice(0, 64), slice(0, 64)),
                  identity_bf[0:64, 0:64]),
                 ((slice(0, 128), jc * 128), jc,
                  (slice(0, 128), slice(64, 192)),
                  identity_bf[:, :])]
        pieces = []
        for (rs, c0), bib, (brs, bcs), rhs in specs:
            if bib < 0 or bib >= NB or not (i0 <= c0 < i0 + 1024):
                continue
            pieces.append(((rs, c0), B[(h, bib)][brs, bcs], rhs))
        return pieces

    def emit_scores_half(h, jc, half, et):
        hs = h * DH
        i0 = half * 1024
        hp = _score_pieces(h, jc, i0)
        piece_banks = {(p[0][1] - i0) // 512 for p in hp}
        ps = ps_s_pool.tile([128, 1024], F32, tag="pss",
                            name=f"pss{h}_{jc}_{half}")
        for qi in range(2):
            nc.tensor.matmul(
                ps[:, qi * 512 : (qi + 1) * 512],
                kTq[jc // 4][hs : hs + DH,
                             (jc % 4) * 128 : (jc % 4 + 1) * 128],
                qTq[2 * half + qi][hs : hs + DH, :],
                start=True,
                stop=(qi not in piece_banks),
                skip_group_check=True,
            )
        remaining = {}
        for p in hp:
            bank = (p[0][1] - i0) // 512
            remaining[bank] = remaining.get(bank, 0) + 1
        for (rs, c0), lhs, rhs in hp:
            bank = (c0 - i0) // 512
            remaining[bank] -= 1
            nc.tensor.matmul(
                ps[rs, c0 - i0 : c0 - i0 + (128 if rs == slice(0, 128) else 64)],
                lhs, rhs,
                start=False, stop=(remaining[bank] == 0),
                skip_group_check=True,
            )
        nc.scalar.activation(et[:, i0 : i0 + 1024], ps[:, :], AF.Exp,
                             scale=SCALE)

    def emit_scores(h, jc):
        et = et_pool.tile([128, 2048], BF16, tag="et", name=f"et{h}_{jc}")
        for half in range(2):
            emit_scores_half(h, jc, half, et)
        return et

    out_sb = [out_pool.tile([128, 4 * DPC], F32, tag="outsb", name=f"outsb{i}")
              for i in range(4)]

    ctxp = {}

    def emit_ctx_half1(h, ic, ets):
        """first 8 j-blocks -> partial ctx parked in SBUF bf16"""
        ps_c = ctx_pool.tile([128, 65], F32, tag="ctx", name=f"cx1_{h}_{ic}")
        for jb in range(8):
            nc.tensor.matmul(
                ps_c[:, :],
                ets[jb][:, ic * 128 : (ic + 1) * 128],
                v_sb[jb][:, h * 65 : h * 65 + 65],
                start=(jb == 0),
                stop=(jb == 7),
                skip_group_check=True,
            )
        cp = ctxp_pool.tile([128, 66], BF16, tag="ctxp", name=f"cp{h}_{ic}")
        nc.vector.tensor_copy(cp[:, 0:65], ps_c[:, :])
        ctxp[(h, ic)] = cp

    def emit_ctx_half2(h, ic, ets):
        """last 8 j-blocks + parked half + rel-v, normalize, write out_sb"""
        ps_c = ctx_pool.tile([128, 65], F32, tag="ctx", name=f"cx2_{h}_{ic}")
        for jb in range(8, NB):
            nc.tensor.matmul(
                ps_c[:, :],
                ets[jb][:, ic * 128 : (ic + 1) * 128],
                v_sb[jb][:, h * 65 : h * 65 + 65],
                start=(jb == 8),
                stop=False,
                skip_group_check=True,
            )
        nc.tensor.matmul(ps_c[:, :], identity_bf[:, :],
                         ctxp[(h, ic)][:, 0:65],
                         start=False, stop=False, skip_group_check=True)
        nc.tensor.matmul(ps_c[:, :], UT[(h, ic)], wrv_sb[:, :],
                         start=False, stop=True, skip_group_check=True)
        rcp = small_pool.tile([128, 1], F32, tag="rcp", name=f"rcp{h}_{ic}")
        nc.vector.reciprocal(rcp[:, :], ps_c[:, 64:65])
        nc.vector.tensor_scalar_mul(
            out_sb[ic // 4][:, (ic % 4) * DPC + h * 64 : (ic % 4) * DPC + h * 64 + 64],
            ps_c[:, 0:64],
            rcp[:, :],
        )

    for q in range(2):
        psp = ps_s_pool.tile([128, 1024], F32, tag="pss", name=f"pss_p{q}")
        emit_proj(psp, 0, qTq[q], "q", q)
        emit_proj(psp, 512, kTq[q], "k", q)
    for ib in range(8):
        emit_ak(0, ib)
    et_early = []
    for jc in range(8):
        et = et_pool.tile([128, 2048], BF16, tag="et", name=f"et0_{jc}")
        emit_scores_half(0, jc, 0, et)
        et_early.append(et)
    for q in range(2, 4):
        psq = med_pool.tile([128, 512], F32, tag="med", name=f"pj_q{q}")
        emit_proj(psq, 0, qTq[q], "q", q)
        psk = med_pool.tile([128, 512], F32, tag="med", name=f"pj_k{q}")
        emit_proj(psk, 0, kTq[q], "k", q)
    for ib in range(8, NB):
        emit_ak(0, ib)
    for ib in range(NB):
        emit_ak(1, ib)


    et_h = {0: [], 1: []}
    Uprev = None
    for h in range(HPC):
        for jc in range(NB):
            if h == 0 and jc < 8:
                emit_scores_half(0, jc, 1, et_early[jc])
                et_h[h].append(et_early[jc])
            else:
                et_h[h].append(emit_scores(h, jc))
            if jc % 2 == 0:
                U = emit_band_pair(h, jc // 2)
            else:
                emit_ut_pair(h, (jc - 1) // 2, U)
            if h == 0:
                for vjb in ([0, 1, 2] if jc == 0 else []) + (
                    [jc + 3] if jc + 3 < NB else []
                ):
                    emit_v(vjb)
            else:
                emit_ctx_half2(0, jc, et_h[0])
            if jc >= 8:
                emit_ctx_half1(h, 2 * (jc - 8), et_h[h])
                emit_ctx_half1(h, 2 * (jc - 8) + 1, et_h[h])
    for ic in range(NB):
        emit_ctx_half2(1, ic, et_h[1])
        if ic % 4 == 3:
            q = ic // 4
            dstv = out[q * 512 : (q + 1) * 512, :].rearrange(
                "(s p) d -> p s d", p=128
            )
            nc.sync.dma_start(
                dstv, out_sb[q][:, :].rearrange("p (s d) -> p s d", d=DPC)
            )
    return nc


_CACHED_NC = None


def get_compiled_nc():
    global _CACHED_NC
    if _CACHED_NC is None:
        nc = bacc.Bacc(
            "TRN2", target_bir_lowering=False, debug=False,
            enable_asserts=True, num_devices=NCORES,
        )
        with tile.TileContext(nc) as tc:
            with ExitStack() as ctx:
                build_kernel(nc, tc, ctx)
        nc.compile()
        _CACHED_NC = nc
    return _CACHED_NC


def _pack_w(w):
    """[1024, 128] f32 -> [128, 1024] bf16; packed[p, a*128+d] = w[a*128+p, d]."""
    return np.ascontiguousarray(
        w.reshape(NC8, 128, DPC).transpose(1, 0, 2).reshape(128, NC8 * DPC)
    ).astype(BF)


def make_shared_inputs(W_rel_k, W_rel_v):
    wrkp = np.zeros((128, WPAD), BF)
    wrkp[0:64, 0:WBAND] = W_rel_k.astype(BF)
    wrkp[64:128, 0:WBAND] = W_rel_k.astype(BF)
    wrv_pad = np.zeros((WPAD, 65), np.float32)
    wrv_pad[0:WBAND, 0:64] = W_rel_v
    wrvp = wrv_pad[0:128].astype(BF)

    pp = np.arange(128)[:, None]
    ww = np.arange(WPAD)[None, :]
    idxbias = (pp + ww).astype(np.int16)

    jw = np.arange(512)[None, :]
    idxb_m = np.broadcast_to(jw, (128, 512)) - pp
    idxb_0 = idxb_m.copy()
    idxb_0[np.broadcast_to(jw < 64, idxb_0.shape)] = -1
    idxb_f = idxb_m.copy()
    idxb_f[np.broadcast_to(jw >= 448, idxb_f.shape)] = -1
    return {
        "wrkp": wrkp, "wrvp": wrvp,
        "idxbias": idxbias,
        "idxb_m": idxb_m.astype(np.int16),
        "idxb_0": idxb_0.astype(np.int16),
        "idxb_f": idxb_f.astype(np.int16),
    }


def prep_core_inputs(xb_shared, Wq, Wk, Wv, shared, core):
    sl = slice(core * DPC, (core + 1) * DPC)
    return {
        "xb": xb_shared,
        "wqp": _pack_w(np.asarray(Wq[:, sl], np.float32)),
        "wkp": _pack_w(np.asarray(Wk[:, sl], np.float32)),
        "wvp": _pack_w(np.asarray(Wv[:, sl], np.float32)),
        **shared,
    }


def kernel(
    hidden_states,
    attention_mask,
    Wq,
    bq,
    Wk,
    bk,
    Wv,
    bv,
    W_rel_k,
    W_rel_v,
):
    hidden_states = np.asarray(hidden_states, np.float32)
    attention_mask = np.asarray(attention_mask, np.float32)
    Wq, Wk, Wv = (np.asarray(w, np.float32) for w in (Wq, Wk, Wv))
    bq, bk, bv = (np.asarray(b, np.float32) for b in (bq, bk, bv))
    W_rel_k = np.asarray(W_rel_k, np.float32)
    W_rel_v = np.asarray(W_rel_v, np.float32)

    assert hidden_states.shape == (1, N, HID)
    assert np.all(attention_mask == 1.0), "kernel assumes all-ones mask"
    assert not np.any(bq) and not np.any(bk) and not np.any(bv), (
        "kernel assumes zero qkv biases"
    )

    xb_shared = np.ascontiguousarray(hidden_states[0]).astype(BF)
    shared = make_shared_inputs(W_rel_k, W_rel_v)
    in_maps = [
        prep_core_inputs(xb_shared, Wq, Wk, Wv, shared, c) for c in range(NCORES)
    ]

    nc = get_compiled_nc()
    res = bass_utils.run_bass_kernel_spmd(nc, in_maps, core_ids=list(range(NCORES)))
    cols = [np.asarray(res.results[c]["out"], np.float32) for c in range(NCORES)]
    full = np.concatenate(cols, axis=1)
    return full.reshape(1, N, HID)


# revision 9
# speedup vs baseline: 1.0011x; 1.0011x over previous
"""Trainium2 Bass kernel v2 for nn_BertSelfAttention_43267500540531.

BertSelfAttention with relative-position key bias and relative-position value
aggregation (half-width 64), B=1, N=2048, HID=1024, 16 heads of d_head=64.

Sharding: 16 heads over 8 cores (2 heads/core). Each core gets full
hidden_states (bf16) and its 128-column slices of Wq/Wk/Wv, writes its 128
output columns; host concatenates.

v2 design (all on-chip, no DRAM skew bounce):
  - xT via x-bar DMA transposes; qT/kT projections per 512-quarter
  - scores computed transposed per j-block: sT[j, i] in a 4-bank psum tile
    [128, 2048]; one Exp activation per block reads the whole span
  - rel-k bias: a_k[i, w] (PE) -> gpsimd local_scatter skews each i-chunk
    into B[i', jj] = bias[i, j] -> three stationary-B matmuls against
    identity transpose-add the [j, i] windows into the score psum
  - softmax denominator L via ones-column appended to v
  - ctx reoriented: out[i-block, 65] accumulates 16 stationary-et matmuls
    (moving operand v_aug [128, 65]); normalization by 1/L on DVE writes
    the final [i, d] layout directly (no output transpose)
  - rel-v: band scores recomputed directly in [i, j] orientation (one small
    matmul per i-chunk + bias add via identity matmul), Exp, local_scatter
    to U[i', r], PE-transpose to U^T, two matmuls against W_rel_v chunks
    accumulate into the ctx psum group

The attention_mask is all-ones and the q/k/v biases are zero in this
problem's setup_inputs; both are validated at entry.
"""

import sys
from contextlib import ExitStack

for _p in ("/opt/trn_rl_repo", "/root/.axon_site/_ro/trn_rl_repo"):
    if _p not in sys.path:
        sys.path.append(_p)

import ml_dtypes
import numpy as np

import concourse.bacc as bacc
import concourse.mybir as mybir
import concourse.tile as tile
from concourse import bass_utils, library_config
from concourse.masks import make_identity

F32 = mybir.dt.float32
BF16 = mybir.dt.bfloat16
I16 = mybir.dt.int16
AF = mybir.ActivationFunctionType
BF = ml_dtypes.bfloat16

N = 2048
HID = 1024
DH = 64
HPC = 2
DPC = HPC * DH
NB = N // 128
NC8 = HID // 128
NCORES = 8
WBAND = 129
WPAD = 132
SCALE = 0.125
BW = 260  # bias scatter dst width (i'+w <= 127+131=258)


def build_kernel(nc, tc, ctx: ExitStack):
    xb = nc.dram_tensor("xb", [N, HID], BF16, kind="ExternalInput").ap()
    wqp = nc.dram_tensor("wqp", [128, HID], BF16, kind="ExternalInput").ap()
    wkp = nc.dram_tensor("wkp", [128, HID], BF16, kind="ExternalInput").ap()
    wvp = nc.dram_tensor("wvp", [128, HID], BF16, kind="ExternalInput").ap()
    wrkp = nc.dram_tensor("wrkp", [128, WPAD], BF16, kind="ExternalInput").ap()
    wrvp = nc.dram_tensor("wrvp", [128, 65], BF16, kind="ExternalInput").ap()
    idxbias = nc.dram_tensor("idxbias", [128, WPAD], I16, kind="ExternalInput").ap()
    idxb_m = nc.dram_tensor("idxb_m", [128, 512], I16, kind="ExternalInput").ap()
    idxb_0 = nc.dram_tensor("idxb_0", [128, 512], I16, kind="ExternalInput").ap()
    idxb_f = nc.dram_tensor("idxb_f", [128, 512], I16, kind="ExternalInput").ap()
    out = nc.dram_tensor("out", [N, DPC], F32, kind="ExternalOutput").ap()

    const_pool = ctx.enter_context(tc.tile_pool(name="const", bufs=1))
    qkT_pool = ctx.enter_context(tc.tile_pool(name="qkT", bufs=4))
    v_pool = ctx.enter_context(tc.tile_pool(name="vsb", bufs=NB))
    et_pool = ctx.enter_context(tc.tile_pool(name="expT", bufs=20))
    ctxp_pool = ctx.enter_context(tc.tile_pool(name="ctxp", bufs=32))
    ak_pool = ctx.enter_context(tc.tile_pool(name="ak", bufs=8))
    b_pool = ctx.enter_context(tc.tile_pool(name="bias", bufs=32))
    eb_pool = ctx.enter_context(tc.tile_pool(name="eb", bufs=4))
    u_pool = ctx.enter_context(tc.tile_pool(name="u", bufs=4))
    ut_pool = ctx.enter_context(tc.tile_pool(name="ut", bufs=34))
    out_pool = ctx.enter_context(tc.tile_pool(name="outsb", bufs=4))
    small_pool = ctx.enter_context(tc.tile_pool(name="small", bufs=4))

    ps_s_pool = ctx.enter_context(tc.tile_pool(name="ps_s", bufs=2, space="PSUM"))
    med_pool = ctx.enter_context(tc.tile_pool(name="ps_m", bufs=2, space="PSUM"))
    ctx_pool = ctx.enter_context(tc.tile_pool(name="ps_c", bufs=2, space="PSUM"))

    nc.gpsimd.load_library(library_config.local_scatter)

    identity = const_pool.tile([128, 128], F32, tag="ident")
    make_identity(nc, identity[:, :])
    identity_bf = const_pool.tile([128, 128], BF16, tag="identb")
    nc.vector.tensor_copy(identity_bf[:, :], identity[:, :])
    zero64 = const_pool.tile([128, 128], BF16, tag="z64")
    nc.vector.memset(zero64[:, :], 0.0)
    warm = const_pool.tile([1, 4], F32, tag="warm")
    nc.scalar.activation(warm[:, :], identity[0:1, 0:4], AF.Exp)

    # q/k weights first (needed by the first projection chunks)
    w_bf = {}
    for wname, wsrc in (("q", wqp), ("k", wkp)):
        wb = const_pool.tile([128, HID], BF16, tag=f"w{wname}")
        nc.sync.dma_start(wb[:, :], wsrc[:, :])
        w_bf[wname] = wb

    # xT via x-bar DMA transposes. First j-quarter split per chunk so the
    # first qT pass can start early; remaining 3 quarters in one DMA each.
    xT_ctx = ExitStack()
    xT_pool = xT_ctx.enter_context(tc.tile_pool(name="xT", bufs=NC8))
    xT_t = [[xT_pool.tile([128, 512], BF16, tag=f"xTq{q}", name=f"xT{q}_{i}")
             for i in range(NC8)] for q in range(2)]
    xT_cd = [xT_pool.tile([128, 1024], BF16, tag="xTcd", name=f"xTcd{i}")
             for i in range(NC8)]
    wrk_bf = const_pool.tile([128, WPAD], BF16, tag="wrk")
    nc.sync.dma_start(wrk_bf[:, :], wrkp[:, :])
    ixb = const_pool.tile([128, WPAD], I16, tag="ixb")
    nc.sync.dma_start(ixb[:, :], idxbias[:, :])
    for q in range(2):
        for chunk in range(NC8):
            cs = slice(chunk * 128, (chunk + 1) * 128)
            nc.sync.dma_start_transpose(
                xT_t[q][chunk][:, :], xb[q * 512 : (q + 1) * 512, cs]
            )
    for chunk in range(NC8):
        cs = slice(chunk * 128, (chunk + 1) * 128)
        nc.sync.dma_start_transpose(xT_cd[chunk][:, :], xb[1024:N, cs])

    def xT_mov(chunk, c0, c1):
        """moving operand slice of x^T chunk covering token cols [c0, c1)"""
        if c1 <= 1024:
            q = c0 // 512
            assert c1 <= (q + 1) * 512
            return xT_t[q][chunk][:, c0 - q * 512 : c1 - q * 512]
        assert c0 >= 1024
        return xT_cd[chunk][:, c0 - 1024 : c1 - 1024]

    # remaining constant loads (issued behind the xT transposes)
    wb = const_pool.tile([128, HID], BF16, tag="wv")
    nc.sync.dma_start(wb[:, :], wvp[:, :])
    w_bf["v"] = wb
    wrv_sb = const_pool.tile([128, 65], BF16, tag="wrv")
    nc.sync.dma_start(wrv_sb[:, :], wrvp[:, :])
    ixm = const_pool.tile([128, 512], I16, tag="ixm")
    nc.sync.dma_start(ixm[:, :], idxb_m[:, :])
    ix0 = const_pool.tile([128, 512], I16, tag="ix0")
    nc.sync.dma_start(ix0[:, :], idxb_0[:, :])
    ixf = const_pool.tile([128, 512], I16, tag="ixf")
    nc.sync.dma_start(ixf[:, :], idxb_f[:, :])

    # qT / kT quarters
    qTq = [qkT_pool.tile([128, 512], BF16, tag="qT", name=f"qT{q}")
           for q in range(4)]
    kTq = [qkT_pool.tile([128, 512], BF16, tag="kT", name=f"kT{q}")
           for q in range(4)]

    def emit_proj(ps, off, dst, wname, q):
        pslice = ps[:, off : off + 512]
        for chunk in range(NC8):
            nc.tensor.matmul(
                pslice,
                w_bf[wname][:, chunk * 128 : (chunk + 1) * 128],
                xT_mov(chunk, q * 512, (q + 1) * 512),
                start=(chunk == 0),
                stop=(chunk == NC8 - 1),
                skip_group_check=True,
            )
        nc.vector.tensor_copy(dst[:, :], pslice)

    ak_sb = {}
    B = {}

    def emit_ak(h, ib):
        hs = h * DH
        pa = ctx_pool.tile([128, WPAD], F32, tag="ctx", name=f"pa{h}_{ib}")
        nc.tensor.matmul(
            pa[:, :],
            qTq[ib // 4][hs : hs + DH, (ib % 4) * 128 : (ib % 4 + 1) * 128],
            wrk_bf[hs : hs + DH, :],
            start=True,
            stop=True,
        )
        ak = ak_pool.tile([128, WPAD], BF16, tag="ak", name=f"ak{h}_{ib}")
        nc.vector.tensor_copy(ak[:, :], pa[:, :])
        ak_sb[(h, ib)] = ak
        Bt = b_pool.tile([128, BW], BF16, tag="bias", name=f"B{h}_{ib}")
        nc.gpsimd.local_scatter(
            Bt[:, :], ak[:, :], ixb[:, :],
            channels=128, num_elems=BW, num_idxs=WPAD,
        )
        B[(h, ib)] = Bt

    v_sb = []

    def emit_v(jb):
        pv = ctx_pool.tile([128, DPC], F32, tag="ctx", name=f"pv{jb}")
        for chunk in range(NC8):
            nc.tensor.matmul(
                pv[:, :],
                xT_mov(chunk, jb * 128, (jb + 1) * 128),
                w_bf["v"][:, chunk * 128 : (chunk + 1) * 128],
                start=(chunk == 0),
                stop=(chunk == NC8 - 1),
            )
        vt = v_pool.tile([128, 130], BF16, tag="vsb", name=f"vsb{jb}")
        nc.vector.tensor_copy(vt[:, 0:64], pv[:, 0:64])
        nc.vector.tensor_copy(vt[:, 65:129], pv[:, 64:128])
        nc.vector.memset(vt[:, 64:65], 1.0)
        nc.vector.memset(vt[:, 129:130], 1.0)
        v_sb.append(vt)

    def kT_mov(hs, j0, j1):
        """pieces (col, width, ap) of kT rows [hs:hs+64] over cols [j0, j1)"""
        pieces = []
        j = j0
        while j < j1:
            q = j // 512
            je = min(j1, (q + 1) * 512)
            pieces.append(
                (j, je - j, kTq[q][hs : hs + DH, j - q * 512 : je - q * 512])
            )
            j = je
        return pieces

    def emit_band_pair(h, icp):
        """two band windows (ic0, ic0+1) -> one exp -> one scatter U2"""
        hs = h * DH
        ic0 = 2 * icp
        ps_b = med_pool.tile([128, 512], F32, tag="med", name=f"psb{h}_{icp}")
        for sub in range(2):
            ic = ic0 + sub
            base = sub * 256
            w0 = ic * 128 - 64
            edge = ic == 0 or ic == NB - 1
            if edge:
                nc.tensor.matmul(
                    ps_b[:, base : base + 256], zero64[hs : hs + DH, :],
                    kTq[0][hs : hs + DH, 0:256],
                    start=True, stop=False, skip_group_check=True,
                )
            first = not edge
            for j, width, mov in kT_mov(hs, max(0, w0), min(N, w0 + 256)):
                nc.tensor.matmul(
                    ps_b[:, base + j - w0 : base + j - w0 + width],
                    qTq[ic // 4][hs : hs + DH,
                                 (ic % 4) * 128 : (ic % 4 + 1) * 128],
                    mov,
                    start=first,
                    stop=False,
                    skip_group_check=True,
                )
                first = False
            nc.tensor.matmul(
                ps_b[:, base : base + 256], identity_bf[:, :],
                B[(h, ic)][:, 0:256],
                start=False, stop=(sub == 1), skip_group_check=True,
            )
        eb = eb_pool.tile([128, 512], BF16, tag="eb", name=f"eb{h}_{icp}")
        nc.scalar.activation(eb[:, :], ps_b[:, :], AF.Exp, scale=SCALE)
        U = u_pool.tile([128, 512], BF16, tag="u", name=f"u{h}_{icp}")
        ix = ix0 if icp == 0 else (ixf if icp == NB // 2 - 1 else ixm)
        nc.gpsimd.local_scatter(
            U[:, :], eb[:, :], ix[:, :],
            channels=128, num_elems=512, num_idxs=512,
        )
        return U

    UT = {}

    def emit_ut_pair(h, icp, U):
        pt = med_pool.tile([128, 256], BF16, tag="med", name=f"pt{h}_{icp}")
        nc.tensor.matmul(pt[:, 0:128], U[:, 0:128], identity_bf[:, :],
                         is_transpose=True)
        nc.tensor.matmul(pt[:, 128:256], U[:, 256:384], identity_bf[:, :],
                         is_transpose=True)
        ut = ut_pool.tile([128, 256], BF16, tag="ut", name=f"ut{h}_{icp}")
        nc.vector.tensor_copy(ut[:, :], pt[:, :])
        UT[(h, 2 * icp)] = ut[:, 0:128]
        UT[(h, 2 * icp + 1)] = ut[:, 128:256]

    UT = {}

    def emit_ut(h, ic, U):
        pt = med_pool.tile([128, 128], BF16, tag="med", name=f"pt{h}_{ic}")
        nc.tensor.matmul(pt[:, 0:128], U[:, 0:128], identity_bf[:, :],
                         is_transpose=True)
        ut = ut_pool.tile([128, 128], BF16, tag="ut", name=f"ut{h}_{ic}")
        nc.vector.tensor_copy(ut[:, :], pt[:, 0:128])
        UT[(h, ic)] = ut

    def _score_pieces(h, jc, i0):
        """bias pieces for jc whose target col lies in [i0, i0+1024)"""
        specs = [((slice(0, 64), (jc - 1) * 128 + 64), jc - 1,
                  (slice(64, 128), slice(192, 256)),
                  identity_bf[64:128, 64:128]),
                 ((slice(64, 128), (jc + 1) * 128), jc + 1,
                  (slice(0, 64), slice(0, 64)),
                  identity_bf[0:64, 0:64]),
                 ((slice(0, 128), jc * 128), jc,
                  (slice(0, 128), slice(64, 192)),
                  identity_bf[:, :])]
        pieces = []
        for (rs, c0), bib, (brs, bcs), rhs in specs:
            if bib < 0 or bib >= NB or not (i0 <= c0 < i0 + 1024):
                continue
            pieces.append(((rs, c0), B[(h, bib)][brs, bcs], rhs))
        return pieces

    def emit_scores_half(h, jc, half, et):
        hs = h * DH
        i0 = half * 1024
        hp = _score_pieces(h, jc, i0)
        piece_banks = {(p[0][1] - i0) // 512 for p in hp}
        ps = ps_s_pool.tile([128, 1024], F32, tag="pss",
                            name=f"pss{h}_{jc}_{half}")
        for qi in range(2):
            nc.tensor.matmul(
                ps[:, qi * 512 : (qi + 1) * 512],
                kTq[jc // 4][hs : hs + DH,
                             (jc % 4) * 128 : (jc % 4 + 1) * 128],
                qTq[2 * half + qi][hs : hs + DH, :],
                start=True,
                stop=(qi not in piece_banks),
                skip_group_check=True,
            )
        remaining = {}
        for p in hp:
            bank = (p[0][1] - i0) // 512
            remaining[bank] = remaining.get(bank, 0) + 1
        for (rs, c0), lhs, rhs in hp:
            bank = (c0 - i0) // 512
            remaining[bank] -= 1
            nc.tensor.matmul(
                ps[rs, c0 - i0 : c0 - i0 + (128 if rs == slice(0, 128) else 64)],
                lhs, rhs,
                start=False, stop=(remaining[bank] == 0),
                skip_group_check=True,
            )
        nc.scalar.activation(et[:, i0 : i0 + 1024], ps[:, :], AF.Exp,
                             scale=SCALE)

    def emit_scores(h, jc):
        et = et_pool.tile([128, 2048], BF16, tag="et", name=f"et{h}_{jc}")
        for half in range(2):
            emit_scores_half(h, jc, half, et)
        return et

    out_sb = [out_pool.tile([128, 4 * DPC], F32, tag="outsb", name=f"outsb{i}")
              for i in range(4)]

    ctxp = {}

    def emit_ctx_half1(h, ic, ets):
        """first 8 j-blocks -> partial ctx parked in SBUF bf16"""
        ps_c = ctx_pool.tile([128, 65], F32, tag="ctx", name=f"cx1_{h}_{ic}")
        for jb in range(8):
            nc.tensor.matmul(
                ps_c[:, :],
                ets[jb][:, ic * 128 : (ic + 1) * 128],
                v_sb[jb][:, h * 65 : h * 65 + 65],
                start=(jb == 0),
                stop=(jb == 7),
                skip_group_check=True,
            )
        cp = ctxp_pool.tile([128, 66], BF16, tag="ctxp", name=f"cp{h}_{ic}")
        nc.vector.tensor_copy(cp[:, 0:65], ps_c[:, :])
        ctxp[(h, ic)] = cp

    def emit_ctx_half2(h, ic, ets):
        """last 8 j-blocks + parked half + rel-v, normalize, write out_sb"""
        ps_c = ctx_pool.tile([128, 65], F32, tag="ctx", name=f"cx2_{h}_{ic}")
        for jb in range(8, NB):
            nc.tensor.matmul(
                ps_c[:, :],
                ets[jb][:, ic * 128 : (ic + 1) * 128],
                v_sb[jb][:, h * 65 : h * 65 + 65],
                start=(jb == 8),
                stop=False,
                skip_group_check=True,
            )
        nc.tensor.matmul(ps_c[:, :], identity_bf[:, :],
                         ctxp[(h, ic)][:, 0:65],
                         start=False, stop=False, skip_group_check=True)
        nc.tensor.matmul(ps_c[:, :], UT[(h, ic)], wrv_sb[:, :],
                         start=False, stop=True, skip_group_check=True)
        rcp = small_pool.tile([128, 1], F32, tag="rcp", name=f"rcp{h}_{ic}")
        nc.vector.reciprocal(rcp[:, :], ps_c[:, 64:65])
        nc.vector.tensor_scalar_mul(
            out_sb[ic // 4][:, (ic % 4) * DPC + h * 64 : (ic % 4) * DPC + h * 64 + 64],
            ps_c[:, 0:64],
            rcp[:, :],
        )

    for q in range(2):
        psp = ps_s_pool.tile([128, 1024], F32, tag="pss", name=f"pss_p{q}")
        emit_proj(psp, 0, qTq[q], "q", q)
        emit_proj(psp, 512, kTq[q], "k", q)
    for ib in range(8):
        emit_ak(0, ib)
    et_early = []
    for jc in range(8):
        et = et_pool.tile([128, 2048], BF16, tag="et", name=f"et0_{jc}")
        emit_scores_half(0, jc, 0, et)
        et_early.append(et)
    for q in range(2, 4):
        psq = med_pool.tile([128, 512], F32, tag="med", name=f"pj_q{q}")
        emit_proj(psq, 0, qTq[q], "q", q)
        psk = med_pool.tile([128, 512], F32, tag="med", name=f"pj_k{q}")
        emit_proj(psk, 0, kTq[q], "k", q)
    for ib in range(8, NB):
        emit_ak(0, ib)
    for ib in range(NB):
        emit_ak(1, ib)


    et_h = {0: [], 1: []}
    Uprev = None
    for h in range(HPC):
        for jc in range(NB):
            if h == 0 and jc < 8:
                emit_scores_half(0, jc, 1, et_early[jc])
                et_h[h].append(et_early[jc])
            else:
                et_h[h].append(emit_scores(h, jc))
            if jc % 2 == 0:
                U = emit_band_pair(h, jc // 2)
            else:
                emit_ut_pair(h, (jc - 1) // 2, U)
            if h == 0:
                for vjb in ([0, 1, 2] if jc == 0 else []) + (
                    [jc + 3] if jc + 3 < NB else []
                ):
                    emit_v(vjb)
            else:
                emit_ctx_half2(0, jc, et_h[0])
            if jc >= 8:
                emit_ctx_half1(h, 2 * (jc - 8), et_h[h])
                emit_ctx_half1(h, 2 * (jc - 8) + 1, et_h[h])
    for ic in range(NB):
        emit_ctx_half2(1, ic, et_h[1])
        if ic % 4 == 3:
            q = ic // 4
            dstv = out[q * 512 : (q + 1) * 512, :].rearrange(
                "(s p) d -> p s d", p=128
            )
            nc.sync.dma_start(
                dstv, out_sb[q][:, :].rearrange("p (s d) -> p s d", d=DPC)
            )
    return nc


_CACHED_NC = None


def get_compiled_nc():
    global _CACHED_NC
    if _CACHED_NC is None:
        nc = bacc.Bacc(
            "TRN2", target_bir_lowering=False, debug=False,
            enable_asserts=True, num_devices=NCORES,
        )
        with tile.TileContext(nc) as tc:
            with ExitStack() as ctx:
                build_kernel(nc, tc, ctx)
        nc.compile()
        _CACHED_NC = nc
    return _CACHED_NC


def _pack_w(w):
    """[1024, 128] f32 -> [128, 1024] bf16; packed[p, a*128+d] = w[a*128+p, d]."""
    return np.ascontiguousarray(
        w.reshape(NC8, 128, DPC).transpose(1, 0, 2).reshape(128, NC8 * DPC)
    ).astype(BF)


def make_shared_inputs(W_rel_k, W_rel_v):
    wrkp = np.zeros((128, WPAD), BF)
    wrkp[0:64, 0:WBAND] = W_rel_k.astype(BF)
    wrkp[64:128, 0:WBAND] = W_rel_k.astype(BF)
    wrv_pad = np.zeros((WPAD, 65), np.float32)
    wrv_pad[0:WBAND, 0:64] = W_rel_v
    wrvp = wrv_pad[0:128].astype(BF)

    pp = np.arange(128)[:, None]
    ww = np.arange(WPAD)[None, :]
    idxbias = (pp + ww).astype(np.int16)

    jw = np.arange(512)[None, :]
    idxb_m = np.broadcast_to(jw, (128, 512)) - pp
    idxb_0 = idxb_m.copy()
    idxb_0[np.broadcast_to(jw < 64, idxb_0.shape)] = -1
    idxb_f = idxb_m.copy()
    idxb_f[np.broadcast_to(jw >= 448, idxb_f.shape)] = -1
    return {
        "wrkp": wrkp, "wrvp": wrvp,
        "idxbias": idxbias,
        "idxb_m": idxb_m.astype(np.int16),
        "idxb_0": idxb_0.astype(np.int16),
        "idxb_f": idxb_f.astype(np.int16),
    }


def prep_core_inputs(xb_shared, Wq, Wk, Wv, shared, core):
    sl = slice(core * DPC, (core + 1) * DPC)
    return {
        "xb": xb_shared,
        "wqp": _pack_w(np.asarray(Wq[:, sl], np.float32)),
        "wkp": _pack_w(np.asarray(Wk[:, sl], np.float32)),
        "wvp": _pack_w(np.asarray(Wv[:, sl], np.float32)),
        **shared,
    }


def kernel(
    hidden_states,
    attention_mask,
    Wq,
    bq,
    Wk,
    bk,
    Wv,
    bv,
    W_rel_k,
    W_rel_v,
):
    hidden_states = np.asarray(hidden_states, np.float32)
    attention_mask = np.asarray(attention_mask, np.float32)
    Wq, Wk, Wv = (np.asarray(w, np.float32) for w in (Wq, Wk, Wv))
    bq, bk, bv = (np.asarray(b, np.float32) for b in (bq, bk, bv))
    W_rel_k = np.asarray(W_rel_k, np.float32)
    W_rel_v = np.asarray(W_rel_v, np.float32)

    assert hidden_states.shape == (1, N, HID)
    assert np.all(attention_mask == 1.0), "kernel assumes all-ones mask"
    assert not np.any(bq) and not np.any(bk) and not np.any(bv), (
        "kernel assumes zero qkv biases"
    )

    xb_shared = np.ascontiguousarray(hidden_states[0]).astype(BF)
    shared = make_shared_inputs(W_rel_k, W_rel_v)
    in_maps = [
        prep_core_inputs(xb_shared, Wq, Wk, Wv, shared, c) for c in range(NCORES)
    ]

    nc = get_compiled_nc()
    res = bass_utils.run_bass_kernel_spmd(nc, in_maps, core_ids=list(range(NCORES)))
    cols = [np.asarray(res.results[c]["out"], np.float32) for c in range(NCORES)]
    full = np.concatenate(cols, axis=1)
    return full.reshape(1, N, HID)
